# revision 37
# baseline (speedup 1.0000x reference)
"""GCN encoder (2-layer GCNConv) as a Bass/Tile kernel on 8 Trainium2 NeuronCores.

Strategy (matches the sharding hint):
  - Nodes row-partitioned across 8 cores (6250 rows each); weights replicated.
  - Symmetric normalization factorized: z = D^-1/2 (A+I) D^-1/2 (x W) + b
    =>  u = dinv * (x W);  agg[d] = u[d] + sum_{e:dst=d} u[src_e];
        z = dinv * agg + b
    so no per-edge norm gather is needed.
  - Per layer: local matmul -> row scale -> AllGather(u) -> per-core gather of
    source rows (dma_gather) -> segment-sum via tensor-engine matmuls with
    compile-time-structured 0/1 selection matrices generated on DVE
    (is_equal against an iota) -> scale/bias/relu -> output rows.
  - Edges are bucketed host-side by (dst window of 128, src range) and padded
    to 128-slot tiles; padded slots gather row 0 and have an all-zero
    selection column, so they contribute nothing.

Perf structure (first working version was 2.22 ms):
  - dma_gather descriptor generation runs on ONE Q7 core pair selected by
    queue_num; with num_swdge_queues=4, four pairs generate concurrently
    (the dominant cost: ~9.3 ns per gathered row per pair).  queue_num is
    assigned AFTER scheduling (BIR position % 4) because Tile's SWDGE
    completion-semaphore lanes are assigned round-robin over the scheduled
    order, and a lane must only ever be incremented by one queue.
  - Each layer's AllGather is split into two window-aligned node-range
    collectives (a = first SPLITW windows, b = rest), and each layer's
    aggregation runs in two passes: pass a consumes only range-a sources
    (partial sums spilled to SBUF in bf16), pass b reloads the partials and
    adds range-b sources.  This keeps the range-b collective entirely off
    the critical path (no head-of-line blocking of the Pool sequencer by a
    gather waiting on an unfinished collective).  Each range also stays
    int16-addressable after cross-rank concat, removing the separate
    src-half split of the first version.
  - The layer-2 transform (h1 @ W2) is fused per-window into layer-1's
    pass b, so the layer-2 range-a collective fires mid-aggregation and
    overlaps remaining layer-1 work.
  - u stores are batched into one DMA per range; large index/selector loads
    are issued after phase A so they don't delay its x-chunk streaming.
  - Selection matrices for all tiles of a (window, range) group are generated
    by a single DVE is_equal over a [128, t, 128] broadcast AP.
  - Slot padding gathers row 0: the SWDGE decoder reserves ring space from
    num_idxs_reg while the Q7 generator trims trailing -1 indices, so -1
    padding leaks ring space and deadlocks await_space; 0-padding keeps both
    sides consistent (the Q7 loop is vectorized per 128-chunk, so sub-tile
    trimming would save no generation time anyway).
"""

import math
import os
import sys

import numpy as np

sys.path.insert(0, "/opt/trn_rl_repo")

import ml_dtypes

BF16 = ml_dtypes.bfloat16

NQ = int(os.environ.get("GCN_NQ", "4"))  # SWDGE queues used round-robin


class Cfg:
    def __init__(self, N, E, IN=512, HID=256, OUT=128, P=8):
        self.N, self.E, self.IN, self.HID, self.OUT, self.P = N, E, IN, HID, OUT, P
        self.NC = N // P                      # nodes per core
        self.WS = 128                         # dst window size
        self.NW = math.ceil(self.NC / self.WS)  # windows per core
        # split local rows into two window-aligned ranges; each range's
        # cross-rank concat must stay int16-addressable (< 32768 rows)
        self.SPLITW = (self.NW + 1) // 2
        self.NCA = min(self.SPLITW * self.WS, self.NC)
        self.NCB = self.NC - self.NCA
        assert P * self.NCA <= 32767 and P * self.NCB <= 32767


FULL = Cfg(N=50000, E=800000)


def _build_smat(aco_arr):
    """Expand per-slot destination columns into 0/1 selection matrices.

    Output [128, tiles, 128]: smat[p, t, j] = 1 iff aco[t*128+p] == j.
    Streamed from DRAM instead of generated on DVE: the generation was a
    2-port DVE op per group, which locks GPSIMD out of its shared SBUF port
    and stalls gather descriptor generation.
    """
    aco = aco_arr.reshape(-1, 128)          # [tiles, slot(partition)]
    tiles = aco.shape[0]
    s = np.zeros((tiles, 128, 128), dtype=BF16)
    tt, pp = np.nonzero(aco >= 0)
    s[tt, pp, aco[tt, pp].astype(np.int64)] = 1
    return np.ascontiguousarray(s.transpose(1, 0, 2))


def _prepare(cfg, x, edge_index, W1, b1, W2, b2):
    """Host-side graph preprocessing -> per-core input maps + program params.

    Slot/selector layout is range-major: all (window, range=a) buckets first,
    then all (window, range=b) buckets, matching the two-pass aggregation.
    """
    N, P, NC, WS, NW = cfg.N, cfg.P, cfg.NC, cfg.WS, cfg.NW
    NCA, NCB = cfg.NCA, cfg.NCB
    src = np.asarray(edge_index[0], dtype=np.int64)
    dst = np.asarray(edge_index[1], dtype=np.int64)

    deg = np.bincount(dst, minlength=N).astype(np.float64) + 1.0  # + self loop
    dinv = (1.0 / np.sqrt(deg)).astype(np.float32)

    # src row ids inside the two all-gathered range tensors
    s_own = src // NC
    s_loc = src % NC
    half = (s_loc >= NCA).astype(np.int64)
    s_row = np.where(half == 0, s_own * NCA + s_loc,
                     s_own * NCB + (s_loc - NCA))

    # group id: (src-range, core, window) ; groups contiguous after sort
    win_id = (dst // NC) * NW + (dst % NC) // WS
    comp = half * (P * NW) + win_id
    order = np.argsort(comp, kind="stable")
    r_s, d_s, c_s = s_row[order], dst[order], comp[order]
    counts = np.bincount(c_s, minlength=2 * P * NW).reshape(2, P, NW)

    # shared tile counts per (range, window): max over cores
    T = np.ceil(counts.max(axis=1) / 128).astype(np.int64)  # [2, NW]
    tiles_total = int(T.sum())
    slots_total = tiles_total * 128

    starts = np.zeros(2 * P * NW + 1, dtype=np.int64)
    np.cumsum(counts.reshape(-1), out=starts[1:])

    in_maps = []
    for c in range(P):
        idx_arr = np.zeros(slots_total, dtype=np.int16)
        aco_arr = np.full(slots_total, -1, dtype=np.float32)
        off = 0
        for h in range(2):
            for w in range(NW):
                t_wh = int(T[h, w])
                if t_wh == 0:
                    continue
                g = (h * P + c) * NW + w
                n = counts[h, c, w]
                sl = slice(starts[g], starts[g] + n)
                idx_arr[off:off + n] = r_s[sl].astype(np.int16)
                aco_arr[off:off + n] = (d_s[sl] - c * NC - w * WS).astype(np.float32)
                off += 128 * t_wh
        assert off == slots_total

        dloc = np.concatenate(
            [dinv[c * NC:(c + 1) * NC],
             np.ones(NW * WS - NC, dtype=np.float32)])

        m = {
            "xT": np.ascontiguousarray(
                np.asarray(x[c * NC:(c + 1) * NC], np.float32).astype(BF16).T
                .reshape(cfg.IN // 128, 128, NC).transpose(1, 0, 2)),
            "w1": np.ascontiguousarray(
                np.asarray(W1, np.float32).astype(BF16)
                .reshape(cfg.IN // 128, 128, cfg.HID).transpose(1, 0, 2)),
            "w2": np.ascontiguousarray(
                np.asarray(W2, np.float32).astype(BF16)
                .reshape(cfg.HID // 128, 128, cfg.OUT).transpose(1, 0, 2)),
            "dinvc": np.ascontiguousarray(dloc.reshape(NW, WS).T),
            "idx": np.ascontiguousarray(np.tile(idx_arr.reshape(-1, 16).T, (8, 1))),
            "smat": _build_smat(aco_arr),
            "ident": np.eye(128, dtype=BF16),
        }
        b1nz = bool(np.any(np.asarray(b1)))
        b2nz = bool(np.any(np.asarray(b2)))
        if b1nz:
            m["b1bc"] = np.ascontiguousarray(
                np.broadcast_to(np.asarray(b1, np.float32), (128, cfg.HID)))
        if b2nz:
            m["b2bc"] = np.ascontiguousarray(
                np.broadcast_to(np.asarray(b2, np.float32), (128, cfg.OUT)))
        in_maps.append(m)

    return in_maps, T, b1nz, b2nz


def build_program(cfg, T, b1nz, b2nz):
    import concourse.bacc as bacc
    import concourse.mybir as mybir
    from concourse import tile

    N, P, NC, WS, NW = cfg.N, cfg.P, cfg.NC, cfg.WS, cfg.NW
    NCA, NCB, SPLITW = cfg.NCA, cfg.NCB, cfg.SPLITW
    IN, HID, OUT = cfg.IN, cfg.HID, cfg.OUT
    NCI, NCH = IN // 128, HID // 128
    tiles_total = int(T.sum())
    slots_total = tiles_total * 128
    TMAX = int(T.max())
    f32, bf16, i16 = mybir.dt.float32, mybir.dt.bfloat16, mybir.dt.int16
    AF = mybir.ActivationFunctionType

    nc = bacc.Bacc("TRN2", target_bir_lowering=False, debug=False,
                   num_devices=cfg.P, num_swdge_queues=NQ)
    xT_p = nc.dram_tensor("xT", [128, NCI, NC], bf16, kind="ExternalInput")
    w1_p = nc.dram_tensor("w1", [128, NCI, HID], bf16, kind="ExternalInput")
    w2_p = nc.dram_tensor("w2", [128, NCH, OUT], bf16, kind="ExternalInput")
    dinv_p = nc.dram_tensor("dinvc", [WS, NW], f32, kind="ExternalInput")
    idx_p = nc.dram_tensor("idx", [128, slots_total // 16], i16, kind="ExternalInput")
    smat_p = nc.dram_tensor("smat", [128, tiles_total, 128], bf16,
                            kind="ExternalInput")
    id_p = nc.dram_tensor("ident", [128, 128], bf16, kind="ExternalInput")
    b1_p = (nc.dram_tensor("b1bc", [128, HID], f32, kind="ExternalInput")
            if b1nz else None)
    b2_p = (nc.dram_tensor("b2bc", [128, OUT], f32, kind="ExternalInput")
            if b2nz else None)
    out_p = nc.dram_tensor("out", [NC, OUT], f32, kind="ExternalOutput")

    u1da = nc.dram_tensor("u1da", [NCA, HID], bf16)
    u1db = nc.dram_tensor("u1db", [NCB, HID], bf16)
    u2da = nc.dram_tensor("u2da", [NCA, OUT], bf16)
    u2db = nc.dram_tensor("u2db", [NCB, OUT], bf16)
    U1a = nc.dram_tensor("U1a", [P * NCA, HID], bf16, addr_space="Shared")
    U1b = nc.dram_tensor("U1b", [P * NCB, HID], bf16, addr_space="Shared")
    U2a = nc.dram_tensor("U2a", [P * NCA, OUT], bf16, addr_space="Shared")
    U2b = nc.dram_tensor("U2b", [P * NCB, OUT], bf16, addr_space="Shared")
    warm_i = nc.dram_tensor("warm_i", [128, 64], bf16)
    warm_o = nc.dram_tensor("warm_o", [P * 128, 64], bf16, addr_space="Shared")
    rg = [list(range(P))]

    # slot/tile offsets of the two range sections (host layout is
    # range-major; per-layer gathers reuse the same sections)
    a_tiles = int(T[0].sum())
    sect_tile0 = {0: 0, 1: a_tiles}
    sect_slot0 = {0: 0, 1: a_tiles * 128}

    with tile.TileContext(nc) as tc:
        with (
            tc.tile_pool(name="res", bufs=1) as res,
            tc.tile_pool(name="work", bufs=4) as work,
            tc.tile_pool(name="gath", bufs=3) as gath,
            tc.tile_pool(name="psum", bufs=2, space="PSUM") as psum,
        ):
            # tiny dummy collective issued first: absorbs the one-time
            # collectives init barrier (~40-50us) under phase A instead of
            # delaying the first real AllGather (collectives must use
            # internal DRAM tensors, so zero the source on device first)
            wz = work.tile([128, 64], bf16, tag="wz")
            nc.gpsimd.memset(wz[:], 0.0)
            nc.sync.dma_start(warm_i[:], wz[:])
            nc.gpsimd.collective_compute(
                "AllGather", mybir.AluOpType.bypass, replica_groups=rg,
                ins=[warm_i[:]], outs=[warm_o[:]])

            # ---- resident loads needed by phase A ----
            w1s = res.tile([128, NCI, HID], bf16)
            nc.sync.dma_start(w1s[:], w1_p[:])
            w2s = res.tile([128, NCH, OUT], bf16)
            nc.sync.dma_start(w2s[:], w2_p[:])
            dinvs = res.tile([WS, NW], f32)
            nc.sync.dma_start(dinvs[:], dinv_p[:])
            ident = res.tile([128, 128], bf16)
            nc.sync.dma_start(ident[:], id_p[:])
            b1bc = None
            if b1nz:
                b1bc = res.tile([128, HID], f32)
                nc.sync.dma_start(b1bc[:], b1_p[:])
            b2bc = None
            if b2nz:
                b2bc = res.tile([128, OUT], f32)
                nc.sync.dma_start(b2bc[:], b2_p[:])

            # hoist gather-count registers (one per distinct tile count) so
            # each dma_gather doesn't spend a Pool MOVE slot on its count
            cnt_regs = {}
            for t_wh in sorted(set(int(t) for t in T.reshape(-1) if t)):
                cnt_regs[t_wh] = nc.gpsimd.to_reg(128 * t_wh)

            u1res = res.tile([128, NW, HID], bf16)
            u2res = res.tile([128, NW, OUT], bf16)
            part = res.tile([128, NW, HID], bf16)   # pass-a partial sums
            h1T = res.tile([128, NCH, NC], bf16)
            idxs = res.tile([128, slots_total // 16], i16)
            if NC % WS:
                # tail rows of the last window feed self-loop/reload matmuls
                # as rhs; zero them so uninitialized SBUF can't inject NaNs
                nc.gpsimd.memset(u1res[:, NW - 1, :], 0.0)
                nc.gpsimd.memset(u2res[:, NW - 1, :], 0.0)
                nc.gpsimd.memset(part[:, NW - 1, :], 0.0)

            def nsz(j):
                return min(128, NC - j * WS)

            MAXP = int(os.environ.get("GCN_MAX_PHASE", "9"))

            def emit_debug_out(src_bf16_ap, w, n):
                # convert [n, OUT] bf16 -> f32, dump into out rows of window w
                dt = work.tile([128, OUT], f32, tag="dbg")
                nc.scalar.activation(dt[:n, :], src_bf16_ap, AF.Copy)
                nc.sync.dma_start(out_p[w * WS:w * WS + n, :], dt[:n, :])

            def store_rng(ud, ures, rng_a):
                """Batched store of a window range of ures into ud."""
                if rng_a:
                    w0, rows = 0, NCA
                else:
                    w0, rows = SPLITW, NCB
                nfull = rows // WS
                tail = rows - nfull * WS
                if nfull:
                    dst = ud[0:nfull * WS, :].rearrange(
                        "(w p) f -> p w f", p=WS)
                    nc.sync.dma_start(dst, ures[:, w0:w0 + nfull, :])
                if tail:
                    nc.sync.dma_start(ud[nfull * WS:, :],
                                      ures[:tail, w0 + nfull, :])

            call_no = 0

            def gather_tiles(U, F, h, w, slot_off):
                """Issue the gather for bucket (range h, window w)."""
                nonlocal call_no
                t_wh = int(T[h, w])
                q = call_no % NQ
                call_no += 1
                g = gath.tile([128, TMAX, F], bf16, tag="g%d" % q)
                nc.gpsimd.dma_gather(
                    g[:, :t_wh, :], U[:],
                    idxs[:, slot_off // 16:(slot_off + 128 * t_wh) // 16],
                    num_idxs=128 * t_wh, num_idxs_reg=cnt_regs[t_wh],
                    elem_size=F, single_packet=False, queue_num=q)
                return g

            def sgen(tile_idx, t_wh):
                S = work.tile([128, TMAX, 128], bf16, tag="S")
                nc.sync.dma_start(S[:, :t_wh, :],
                                  smat_p[:, tile_idx:tile_idx + t_wh, :])
                return S

            # ---- phase A: t1 = x @ W1 ; u1 = dinv * t1 ; split AllGather ----
            for j in range(NW):
                n = nsz(j)
                jsl = slice(j * WS, j * WS + n)
                xc = work.tile([128, NCI, WS], bf16, tag="xc")
                nc.sync.dma_start(xc[:, :, :n], xT_p[:, :, jsl])
                pt = psum.tile([128, HID], f32, tag="mm")
                for ci in range(NCI):
                    nc.tensor.matmul(pt[:n, :], xc[:, ci, :n],
                                     w1s[:, ci, :], start=(ci == 0),
                                     stop=(ci == NCI - 1))
                nc.scalar.activation(u1res[:n, j, :], pt[:n, :], AF.Copy,
                                     scale=dinvs[:n, j:j + 1])
                if MAXP == 1:
                    emit_debug_out(u1res[:n, j, :OUT], j, n)
                if j == SPLITW - 1:
                    store_rng(u1da, u1res, True)
                    nc.gpsimd.collective_compute(
                        "AllGather", mybir.AluOpType.bypass,
                        replica_groups=rg, ins=[u1da[:]], outs=[U1a[:]])
            store_rng(u1db, u1res, False)
            if MAXP <= 1:
                return nc
            nc.gpsimd.collective_compute(
                "AllGather", mybir.AluOpType.bypass, replica_groups=rg,
                ins=[u1db[:]], outs=[U1b[:]])

            # big constant loads deferred here so they don't delay phase A's
            # x-chunk streaming on the Sync DMA queue
            nc.sync.dma_start(idxs[:], idx_p[:])

            def pass_a(Ua, F, ures):
                """Aggregate self-loop + range-a sources into part (bf16)."""
                tile_idx = sect_tile0[0]
                slot_off = sect_slot0[0]
                for w in range(NW):
                    n = nsz(w)
                    t_wh = int(T[0, w])
                    pa = psum.tile([128, F], f32, tag="agg")
                    nc.tensor.matmul(pa[:n, :], ident[:, :n], ures[:, w, :],
                                     start=True, stop=(t_wh == 0))
                    if t_wh:
                        g = gather_tiles(Ua, F, 0, w, slot_off)
                        slot_off += 128 * t_wh
                        S = sgen(tile_idx, t_wh)
                        tile_idx += t_wh
                        for t in range(t_wh):
                            nc.tensor.matmul(pa[:n, :], S[:, t, :n],
                                             g[:, t, :], start=False,
                                             stop=(t == t_wh - 1))
                    nc.scalar.activation(part[:n, w, :F], pa[:n, :], AF.Copy)

            def pass_b(Ub, F, bbc, relu, emit_out):
                """Reload partials, add range-b sources, finish z."""
                tile_idx = sect_tile0[1]
                slot_off = sect_slot0[1]
                for w in range(NW):
                    n = nsz(w)
                    t_wh = int(T[1, w])
                    pa = psum.tile([128, F], f32, tag="agg")
                    nc.tensor.matmul(pa[:n, :], ident[:, :n], part[:, w, :F],
                                     start=True, stop=(t_wh == 0))
                    if t_wh:
                        g = gather_tiles(Ub, F, 1, w, slot_off)
                        slot_off += 128 * t_wh
                        S = sgen(tile_idx, t_wh)
                        tile_idx += t_wh
                        for t in range(t_wh):
                            nc.tensor.matmul(pa[:n, :], S[:, t, :n],
                                             g[:, t, :], start=False,
                                             stop=(t == t_wh - 1))
                    # z = dinv * agg (+ b) ; relu
                    if bbc is None:
                        zf = AF.Relu if relu else AF.Copy
                        zt = work.tile([128, F], f32 if emit_out else bf16,
                                       tag="zt%d" % F)
                        nc.scalar.activation(zt[:n, :], pa[:n, :], zf,
                                             scale=dinvs[:n, w:w + 1])
                    else:
                        v = work.tile([128, F], f32, tag="v%d" % F)
                        nc.scalar.activation(v[:n, :], pa[:n, :], AF.Copy,
                                             scale=dinvs[:n, w:w + 1])
                        zt = work.tile([128, F], f32 if emit_out else bf16,
                                       tag="zt%d" % F)
                        if relu:
                            vb = work.tile([128, F], f32, tag="vb%d" % F)
                            nc.vector.tensor_tensor(
                                vb[:n, :], v[:n, :], bbc[:n, :],
                                op=mybir.AluOpType.add)
                            nc.scalar.activation(zt[:n, :], vb[:n, :], AF.Relu)
                        else:
                            nc.vector.tensor_tensor(
                                zt[:n, :], v[:n, :], bbc[:n, :],
                                op=mybir.AluOpType.add)
                    yield w, n, zt

            # ---- layer 1 aggregation (2 passes) + fused layer-2 transform --
            pass_a(U1a, HID, u1res)
            for w, n, zt in pass_b(U1b, HID, b1bc, True, False):
                wsl = slice(w * WS, w * WS + n)
                for ch in range(NCH):
                    ptr = psum.tile([128, 128], bf16, tag="tr")
                    nc.tensor.transpose(ptr[:, :n],
                                        zt[:n, ch * 128:(ch + 1) * 128],
                                        ident[:n, :n])
                    nc.scalar.activation(h1T[:, ch, wsl], ptr[:, :n], AF.Copy)
                if MAXP == 3:
                    emit_debug_out(zt[:n, :OUT], w, n)
                    continue
                # fused phase D: u2[w] = dinv * (h1[w] @ W2)
                pt = psum.tile([128, OUT], f32, tag="mm")
                for ch in range(NCH):
                    nc.tensor.matmul(pt[:n, :], h1T[:, ch, wsl],
                                     w2s[:, ch, :], start=(ch == 0),
                                     stop=(ch == NCH - 1))
                nc.scalar.activation(u2res[:n, w, :], pt[:n, :], AF.Copy,
                                     scale=dinvs[:n, w:w + 1])
                if w == SPLITW - 1:
                    store_rng(u2da, u2res, True)
                    nc.gpsimd.collective_compute(
                        "AllGather", mybir.AluOpType.bypass,
                        replica_groups=rg, ins=[u2da[:]], outs=[U2a[:]])
            if MAXP <= 3:
                return nc
            store_rng(u2db, u2res, False)
            nc.gpsimd.collective_compute(
                "AllGather", mybir.AluOpType.bypass, replica_groups=rg,
                ins=[u2db[:]], outs=[U2b[:]])

            # ---- layer 2 aggregation (2 passes) -> out ----
            pass_a(U2a, OUT, u2res)
            for w, n, zt in pass_b(U2b, OUT, b2bc, False, True):
                wsl = slice(w * WS, w * WS + n)
                nc.sync.dma_start(out_p[wsl, :], zt[:n, :])

    return nc


def _assign_gather_queues(nc):
    """Post-schedule queue assignment: queue_num = BIR position % NQ.

    Tile assigns SWDGE DMA-completion semaphore lanes round-robin over the
    *scheduled* order of Pool DMA instructions (lane = pos % 8), ignoring
    queue_num.  Each lane must only ever be incremented by one SWDGE queue
    (decoder shadow-sem rule), so the queue must also be a function of the
    scheduled position: queue = pos % NQ gives queue q the lane set
    {q, q+NQ}.  Build-time round-robin is NOT sufficient because the
    scheduler reorders independent gathers.
    """
    import concourse.mybir as mybir

    pos = 0
    for f in nc.m.functions:
        for bb in f.blocks:
            for inst in bb.instructions:
                if isinstance(inst, mybir.InstDMAGatherAnt):
                    inst.queue_num = pos % NQ
                    pos += 1
                elif (getattr(inst, "engine", None) == mybir.EngineType.Pool
                      and isinstance(inst, (mybir.InstDMACopy,
                                            mybir.InstDMAScatterAddAnt))):
                    raise AssertionError(
                        "unexpected Pool DMA inst would shift SWDGE sem lanes")
    return pos


def run(cfg, inputs, sim=False, trace=False):
    from concourse.bass_utils import run_bass_kernel_spmd

    in_maps, T, b1nz, b2nz = _prepare(
        cfg, inputs["x"], inputs["edge_index"], inputs["W1"], inputs["b1"],
        inputs["W2"], inputs["b2"])
    nc = build_program(cfg, T, b1nz, b2nz)
    nc.finalize()
    _assign_gather_queues(nc)
    core_ids = list(range(cfg.P))
    if sim:
        from concourse import bass_interp
        ms = bass_interp.MultiCoreSim(nc, cfg.P)
        for c in core_ids:
            for k, v in in_maps[c].items():
                ms.cores[c].tensor(k)[:] = v
        ms.simulate()
        outs = [np.array(ms.cores[c].tensor("out")) for c in core_ids]
        return np.concatenate(outs, axis=0), None
    res = run_bass_kernel_spmd(nc, in_maps, core_ids, trace=trace)
    outs = [np.asarray(res.results[c]["out"]) for c in core_ids]
    return np.concatenate(outs, axis=0), res


def kernel(x, edge_index, W1, b1, W2, b2):
    out, _ = run(FULL, dict(x=x, edge_index=edge_index, W1=W1, b1=b1,
                            W2=W2, b2=b2))
    return out


# revision 49
# speedup vs baseline: 1.1420x; 1.1420x over previous
"""GCN encoder (2-layer GCNConv) as a Bass/Tile kernel on 8 Trainium2 NeuronCores.

Strategy (matches the sharding hint):
  - Nodes row-partitioned across 8 cores (6250 rows each); weights replicated.
  - Symmetric normalization factorized: z = D^-1/2 (A+I) D^-1/2 (x W) + b
    =>  u = dinv * (x W);  agg[d] = u[d] + sum_{e:dst=d} u[src_e];
        z = dinv * agg + b
    so no per-edge norm gather is needed.
  - Per layer: local matmul -> row scale -> AllGather(u) -> per-core gather of
    source rows (dma_gather) -> segment-sum via tensor-engine matmuls with
    compile-time-structured 0/1 selection matrices generated on DVE
    (is_equal against an iota) -> scale/bias/relu -> output rows.
  - Edges are bucketed host-side by (dst window of 128, src range) and padded
    to 128-slot tiles; padded slots gather row 0 and have an all-zero
    selection column, so they contribute nothing.

Perf structure (first working version was 2.22 ms):
  - dma_gather descriptor generation runs on ONE Q7 core pair selected by
    queue_num; with num_swdge_queues=4, four pairs generate concurrently
    (the dominant cost: ~9.3 ns per gathered row per pair).  queue_num is
    assigned AFTER scheduling (BIR position % 4) because Tile's SWDGE
    completion-semaphore lanes are assigned round-robin over the scheduled
    order, and a lane must only ever be incremented by one queue.
  - Each layer's AllGather is split into two window-aligned node-range
    collectives (a = first SPLITW windows, b = rest), and each layer's
    aggregation runs in two passes: pass a consumes only range-a sources
    (partial sums spilled to SBUF in bf16), pass b reloads the partials and
    adds range-b sources.  This keeps the range-b collective entirely off
    the critical path (no head-of-line blocking of the Pool sequencer by a
    gather waiting on an unfinished collective).  Each range also stays
    int16-addressable after cross-rank concat, removing the separate
    src-half split of the first version.
  - The layer-2 transform (h1 @ W2) is fused per-window into layer-1's
    pass b, so the layer-2 range-a collective fires mid-aggregation and
    overlaps remaining layer-1 work.
  - u stores are batched into one DMA per range; large index/selector loads
    are issued after phase A so they don't delay its x-chunk streaming.
  - Selection matrices for all tiles of a (window, range) group are generated
    by a single DVE is_equal over a [128, t, 128] broadcast AP.
  - Slot padding gathers row 0: the SWDGE decoder reserves ring space from
    num_idxs_reg while the Q7 generator trims trailing -1 indices, so -1
    padding leaks ring space and deadlocks await_space; 0-padding keeps both
    sides consistent (the Q7 loop is vectorized per 128-chunk, so sub-tile
    trimming would save no generation time anyway).
"""

import math
import os
import sys

import numpy as np

sys.path.insert(0, "/opt/trn_rl_repo")

import ml_dtypes

BF16 = ml_dtypes.bfloat16

NQ = int(os.environ.get("GCN_NQ", "4"))  # SWDGE queues used round-robin


class Cfg:
    def __init__(self, N, E, IN=512, HID=256, OUT=128, P=8):
        self.N, self.E, self.IN, self.HID, self.OUT, self.P = N, E, IN, HID, OUT, P
        self.NC = N // P                      # nodes per core
        self.WS = 128                         # dst window size
        self.NW = math.ceil(self.NC / self.WS)  # windows per core
        # split local rows into two window-aligned ranges; each range's
        # cross-rank concat must stay int16-addressable (< 32768 rows)
        self.SPLITW = (self.NW + 1) // 2
        self.NCA = min(self.SPLITW * self.WS, self.NC)
        self.NCB = self.NC - self.NCA
        assert P * self.NCA <= 32767 and P * self.NCB <= 32767


FULL = Cfg(N=50000, E=800000)





def _prepare(cfg, x, edge_index, W1, b1, W2, b2):
    """Host-side graph preprocessing -> per-core input maps + program params.

    Slot/selector layout is range-major: all (window, range=a) buckets first,
    then all (window, range=b) buckets, matching the two-pass aggregation.
    """
    N, P, NC, WS, NW = cfg.N, cfg.P, cfg.NC, cfg.WS, cfg.NW
    NCA, NCB = cfg.NCA, cfg.NCB
    src = np.asarray(edge_index[0], dtype=np.int64)
    dst = np.asarray(edge_index[1], dtype=np.int64)

    deg = np.bincount(dst, minlength=N).astype(np.float64) + 1.0  # + self loop
    dinv = (1.0 / np.sqrt(deg)).astype(np.float32)

    # src row ids inside the two all-gathered range tensors
    s_own = src // NC
    s_loc = src % NC
    half = (s_loc >= NCA).astype(np.int64)
    s_row = np.where(half == 0, s_own * NCA + s_loc,
                     s_own * NCB + (s_loc - NCA))

    # group id: (src-range, core, window) ; groups contiguous after sort
    win_id = (dst // NC) * NW + (dst % NC) // WS
    comp = half * (P * NW) + win_id
    order = np.argsort(comp, kind="stable")
    r_s, d_s, c_s = s_row[order], dst[order], comp[order]
    counts = np.bincount(c_s, minlength=2 * P * NW).reshape(2, P, NW)

    # shared tile counts per (range, window): max over cores
    T = np.ceil(counts.max(axis=1) / 128).astype(np.int64)  # [2, NW]
    tiles_total = int(T.sum())
    slots_total = tiles_total * 128

    starts = np.zeros(2 * P * NW + 1, dtype=np.int64)
    np.cumsum(counts.reshape(-1), out=starts[1:])

    in_maps = []
    for c in range(P):
        idx_arr = np.zeros(slots_total, dtype=np.int16)
        aco_arr = np.full(slots_total, -1, dtype=np.float32)
        off = 0
        for h in range(2):
            for w in range(NW):
                t_wh = int(T[h, w])
                if t_wh == 0:
                    continue
                g = (h * P + c) * NW + w
                n = counts[h, c, w]
                sl = slice(starts[g], starts[g] + n)
                idx_arr[off:off + n] = r_s[sl].astype(np.int16)
                aco_arr[off:off + n] = (d_s[sl] - c * NC - w * WS).astype(np.float32)
                off += 128 * t_wh
        assert off == slots_total

        dloc = np.concatenate(
            [dinv[c * NC:(c + 1) * NC],
             np.ones(NW * WS - NC, dtype=np.float32)])

        m = {
            "xT": np.ascontiguousarray(
                np.asarray(x[c * NC:(c + 1) * NC], np.float32).astype(BF16).T
                .reshape(cfg.IN // 128, 128, NC).transpose(1, 0, 2)),
            "w1": np.ascontiguousarray(
                np.asarray(W1, np.float32).astype(BF16)
                .reshape(cfg.IN // 128, 128, cfg.HID).transpose(1, 0, 2)),
            "w2": np.ascontiguousarray(
                np.asarray(W2, np.float32).astype(BF16)
                .reshape(cfg.HID // 128, 128, cfg.OUT).transpose(1, 0, 2)),
            "dinvc": np.ascontiguousarray(dloc.reshape(NW, WS).T),
            "idx": np.ascontiguousarray(np.tile(idx_arr.reshape(-1, 16).T, (8, 1))),
            "acol": np.ascontiguousarray(aco_arr.reshape(-1, 128).T.astype(BF16)),
            "ident": np.eye(128, dtype=BF16),
        }
        b1nz = bool(np.any(np.asarray(b1)))
        b2nz = bool(np.any(np.asarray(b2)))
        if b1nz:
            m["b1bc"] = np.ascontiguousarray(
                np.broadcast_to(np.asarray(b1, np.float32), (128, cfg.HID)))
        if b2nz:
            m["b2bc"] = np.ascontiguousarray(
                np.broadcast_to(np.asarray(b2, np.float32), (128, cfg.OUT)))
        in_maps.append(m)

    return in_maps, T, b1nz, b2nz


def build_program(cfg, T, b1nz, b2nz):
    import concourse.bacc as bacc
    import concourse.mybir as mybir
    from concourse import tile

    N, P, NC, WS, NW = cfg.N, cfg.P, cfg.NC, cfg.WS, cfg.NW
    NCA, NCB, SPLITW = cfg.NCA, cfg.NCB, cfg.SPLITW
    IN, HID, OUT = cfg.IN, cfg.HID, cfg.OUT
    NCI, NCH = IN // 128, HID // 128
    tiles_total = int(T.sum())
    slots_total = tiles_total * 128
    TMAX = int(T.max())
    f32, bf16, i16 = mybir.dt.float32, mybir.dt.bfloat16, mybir.dt.int16
    AF = mybir.ActivationFunctionType

    nc = bacc.Bacc("TRN2", target_bir_lowering=False, debug=False,
                   num_devices=cfg.P, num_swdge_queues=NQ)
    xT_p = nc.dram_tensor("xT", [128, NCI, NC], bf16, kind="ExternalInput")
    w1_p = nc.dram_tensor("w1", [128, NCI, HID], bf16, kind="ExternalInput")
    w2_p = nc.dram_tensor("w2", [128, NCH, OUT], bf16, kind="ExternalInput")
    dinv_p = nc.dram_tensor("dinvc", [WS, NW], f32, kind="ExternalInput")
    idx_p = nc.dram_tensor("idx", [128, slots_total // 16], i16, kind="ExternalInput")
    acol_p = nc.dram_tensor("acol", [128, tiles_total], bf16, kind="ExternalInput")
    id_p = nc.dram_tensor("ident", [128, 128], bf16, kind="ExternalInput")
    b1_p = (nc.dram_tensor("b1bc", [128, HID], f32, kind="ExternalInput")
            if b1nz else None)
    b2_p = (nc.dram_tensor("b2bc", [128, OUT], f32, kind="ExternalInput")
            if b2nz else None)
    out_p = nc.dram_tensor("out", [NC, OUT], f32, kind="ExternalOutput")

    u1da = nc.dram_tensor("u1da", [NCA, HID], bf16)
    u1db = nc.dram_tensor("u1db", [NCB, HID], bf16)
    u2da = nc.dram_tensor("u2da", [NCA, OUT], bf16)
    u2db = nc.dram_tensor("u2db", [NCB, OUT], bf16)
    U1a = nc.dram_tensor("U1a", [P * NCA, HID], bf16, addr_space="Shared")
    U1b = nc.dram_tensor("U1b", [P * NCB, HID], bf16, addr_space="Shared")
    U2a = nc.dram_tensor("U2a", [P * NCA, OUT], bf16, addr_space="Shared")
    U2b = nc.dram_tensor("U2b", [P * NCB, OUT], bf16, addr_space="Shared")
    rg = [list(range(P))]

    # slot/tile offsets of the two range sections (host layout is
    # range-major; per-layer gathers reuse the same sections)
    a_tiles = int(T[0].sum())
    sect_tile0 = {0: 0, 1: a_tiles}
    sect_slot0 = {0: 0, 1: a_tiles * 128}

    with tile.TileContext(nc) as tc:
        with (
            tc.tile_pool(name="res", bufs=1) as res,
            tc.tile_pool(name="work", bufs=4) as work,
            tc.tile_pool(name="gath", bufs=4) as gath,
            tc.tile_pool(name="psum", bufs=2, space="PSUM") as psum,
            tc.tile_pool(name="psacc", bufs=3, space="PSUM") as psacc,
        ):
            # ---- resident loads needed by phase A ----
            w1s = res.tile([128, NCI, HID], bf16)
            nc.sync.dma_start(w1s[:], w1_p[:])
            w2s = res.tile([128, NCH, OUT], bf16)
            nc.sync.dma_start(w2s[:], w2_p[:])
            dinvs = res.tile([WS, NW], f32)
            nc.sync.dma_start(dinvs[:], dinv_p[:])
            ident = res.tile([128, 128], bf16)
            nc.sync.dma_start(ident[:], id_p[:])
            iot = res.tile([128, TMAX, 128], bf16)
            nc.gpsimd.iota(iot[:], pattern=[[0, TMAX], [1, 128]], base=0,
                           channel_multiplier=0,
                           allow_small_or_imprecise_dtypes=True)
            b1bc = None
            if b1nz:
                b1bc = res.tile([128, HID], f32)
                nc.sync.dma_start(b1bc[:], b1_p[:])
            b2bc = None
            if b2nz:
                b2bc = res.tile([128, OUT], f32)
                nc.sync.dma_start(b2bc[:], b2_p[:])

            # hoist gather-count registers (one per distinct tile count) so
            # each dma_gather doesn't spend a Pool MOVE slot on its count
            GMAX0 = (TMAX + 1) // 2
            sizes = set()
            for t_wh in set(int(t) for t in T.reshape(-1) if t):
                for t0 in range(0, t_wh, GMAX0):
                    sizes.add(min(GMAX0, t_wh - t0))
            cnt_regs = {}
            for nt in sorted(sizes):
                cnt_regs[nt] = nc.gpsimd.to_reg(128 * nt)

            u1res = res.tile([128, NW, HID], bf16)
            u2res = res.tile([128, NW, OUT], bf16)
            part = res.tile([128, NW, HID], bf16)   # pass-a partial sums
            h1T = res.tile([128, NCH, NC], bf16)
            idxs = res.tile([128, slots_total // 16], i16)
            acols = res.tile([128, tiles_total], bf16)
            if NC % WS:
                # tail rows of the last window feed self-loop/reload matmuls
                # as rhs; zero them so uninitialized SBUF can't inject NaNs
                nc.gpsimd.memset(u1res[:, NW - 1, :], 0.0)
                nc.gpsimd.memset(u2res[:, NW - 1, :], 0.0)
                nc.gpsimd.memset(part[:, NW - 1, :], 0.0)

            def nsz(j):
                return min(128, NC - j * WS)

            MAXP = int(os.environ.get("GCN_MAX_PHASE", "9"))

            def emit_debug_out(src_bf16_ap, w, n):
                # convert [n, OUT] bf16 -> f32, dump into out rows of window w
                dt = work.tile([128, OUT], f32, tag="dbg")
                nc.scalar.activation(dt[:n, :], src_bf16_ap, AF.Copy)
                nc.sync.dma_start(out_p[w * WS:w * WS + n, :], dt[:n, :])

            def store_rng(ud, ures, rng_a):
                """Batched store of a window range of ures into ud."""
                if rng_a:
                    w0, rows = 0, NCA
                else:
                    w0, rows = SPLITW, NCB
                nfull = rows // WS
                tail = rows - nfull * WS
                if nfull:
                    dst = ud[0:nfull * WS, :].rearrange(
                        "(w p) f -> p w f", p=WS)
                    nc.sync.dma_start(dst, ures[:, w0:w0 + nfull, :])
                if tail:
                    nc.sync.dma_start(ud[nfull * WS:, :],
                                      ures[:tail, w0 + nfull, :])

            call_no = 0
            GMAX = (TMAX + 1) // 2  # tiles per (split) gather call

            def gather_tiles(U, F, h, w, slot_off):
                """Issue the gathers for bucket (range h, window w).

                Split into two calls so the Pool pipeline quantum is smaller
                (finer retire pacing across the 4 SWDGE queues).  Returns a
                list of (g_tile, ntiles).
                """
                nonlocal call_no
                t_wh = int(T[h, w])
                out = []
                for t0 in range(0, t_wh, GMAX):
                    nt = min(GMAX, t_wh - t0)
                    q = call_no % NQ
                    call_no += 1
                    g = gath.tile([128, GMAX, F], bf16, tag="g%d" % q)
                    so = slot_off + 128 * t0
                    nc.gpsimd.dma_gather(
                        g[:, :nt, :], U[:],
                        idxs[:, so // 16:(so + 128 * nt) // 16],
                        num_idxs=128 * nt, num_idxs_reg=cnt_regs[nt],
                        elem_size=F, single_packet=False, queue_num=q)
                    out.append((g, nt))
                return out

            def sgen(tile_idx, t_wh):
                S = work.tile([128, TMAX, 128], bf16, tag="S")
                nc.vector.tensor_tensor(
                    S[:, :t_wh, :], iot[:, :t_wh, :],
                    acols[:, tile_idx:tile_idx + t_wh]
                    .broadcast_to((128, t_wh, 128)),
                    op=mybir.AluOpType.is_equal)
                return S

            # ---- phase A: t1 = x @ W1 ; u1 = dinv * t1 ; split AllGather ----
            for j in range(NW):
                n = nsz(j)
                jsl = slice(j * WS, j * WS + n)
                xc = work.tile([128, NCI, WS], bf16, tag="xc")
                nc.sync.dma_start(xc[:, :, :n], xT_p[:, :, jsl])
                pt = psum.tile([128, HID], f32, tag="mm")
                for ci in range(NCI):
                    nc.tensor.matmul(pt[:n, :], xc[:, ci, :n],
                                     w1s[:, ci, :], start=(ci == 0),
                                     stop=(ci == NCI - 1))
                nc.scalar.activation(u1res[:n, j, :], pt[:n, :], AF.Copy,
                                     scale=dinvs[:n, j:j + 1])
                if MAXP == 1:
                    emit_debug_out(u1res[:n, j, :OUT], j, n)
                if j == SPLITW - 1:
                    store_rng(u1da, u1res, True)
                    nc.gpsimd.collective_compute(
                        "AllGather", mybir.AluOpType.bypass,
                        replica_groups=rg, ins=[u1da[:]], outs=[U1a[:]])
            store_rng(u1db, u1res, False)
            if MAXP <= 1:
                return nc
            nc.gpsimd.collective_compute(
                "AllGather", mybir.AluOpType.bypass, replica_groups=rg,
                ins=[u1db[:]], outs=[U1b[:]])

            # big constant loads deferred here so they don't delay phase A's
            # x-chunk streaming on the Sync DMA queue
            nc.sync.dma_start(idxs[:], idx_p[:])
            nc.sync.dma_start(acols[:], acol_p[:])

            def pass_a(Ua, F, ures):
                """Aggregate self-loop + range-a sources into part (bf16)."""
                tile_idx = sect_tile0[0]
                slot_off = sect_slot0[0]
                for w in range(NW):
                    n = nsz(w)
                    t_wh = int(T[0, w])
                    pa = psacc.tile([128, F], f32, tag="agg")
                    nc.tensor.matmul(pa[:n, :], ident[:, :n], ures[:, w, :],
                                     start=True, stop=(t_wh == 0))
                    if t_wh:
                        chunks = gather_tiles(Ua, F, 0, w, slot_off)
                        slot_off += 128 * t_wh
                        S = sgen(tile_idx, t_wh)
                        tile_idx += t_wh
                        done = 0
                        for g, nt in chunks:
                            for t in range(nt):
                                done += 1
                                nc.tensor.matmul(
                                    pa[:n, :], S[:, done - 1, :n],
                                    g[:, t, :], start=False,
                                    stop=(done == t_wh))
                    nc.scalar.activation(part[:n, w, :F], pa[:n, :], AF.Copy)

            def pass_b(Ub, F, bbc, relu, emit_out):
                """Reload partials, add range-b sources, finish z."""
                tile_idx = sect_tile0[1]
                slot_off = sect_slot0[1]
                for w in range(NW):
                    n = nsz(w)
                    t_wh = int(T[1, w])
                    pa = psacc.tile([128, F], f32, tag="agg")
                    nc.tensor.matmul(pa[:n, :], ident[:, :n], part[:, w, :F],
                                     start=True, stop=(t_wh == 0))
                    if t_wh:
                        chunks = gather_tiles(Ub, F, 1, w, slot_off)
                        slot_off += 128 * t_wh
                        S = sgen(tile_idx, t_wh)
                        tile_idx += t_wh
                        done = 0
                        for g, nt in chunks:
                            for t in range(nt):
                                done += 1
                                nc.tensor.matmul(
                                    pa[:n, :], S[:, done - 1, :n],
                                    g[:, t, :], start=False,
                                    stop=(done == t_wh))
                    # z = dinv * agg (+ b) ; relu
                    if bbc is None:
                        zf = AF.Relu if relu else AF.Copy
                        zt = work.tile([128, F], f32 if emit_out else bf16,
                                       tag="zt%d" % F)
                        nc.scalar.activation(zt[:n, :], pa[:n, :], zf,
                                             scale=dinvs[:n, w:w + 1])
                    else:
                        v = work.tile([128, F], f32, tag="v%d" % F)
                        nc.scalar.activation(v[:n, :], pa[:n, :], AF.Copy,
                                             scale=dinvs[:n, w:w + 1])
                        zt = work.tile([128, F], f32 if emit_out else bf16,
                                       tag="zt%d" % F)
                        if relu:
                            vb = work.tile([128, F], f32, tag="vb%d" % F)
                            nc.vector.tensor_tensor(
                                vb[:n, :], v[:n, :], bbc[:n, :],
                                op=mybir.AluOpType.add)
                            nc.scalar.activation(zt[:n, :], vb[:n, :], AF.Relu)
                        else:
                            nc.vector.tensor_tensor(
                                zt[:n, :], v[:n, :], bbc[:n, :],
                                op=mybir.AluOpType.add)
                    yield w, n, zt

            # ---- layer 1 aggregation (2 passes) + fused layer-2 transform --
            pass_a(U1a, HID, u1res)
            for w, n, zt in pass_b(U1b, HID, b1bc, True, False):
                wsl = slice(w * WS, w * WS + n)
                for ch in range(NCH):
                    ptr = psum.tile([128, 128], bf16, tag="tr")
                    nc.tensor.transpose(ptr[:, :n],
                                        zt[:n, ch * 128:(ch + 1) * 128],
                                        ident[:n, :n])
                    nc.scalar.activation(h1T[:, ch, wsl], ptr[:, :n], AF.Copy)
                if MAXP == 3:
                    emit_debug_out(zt[:n, :OUT], w, n)
                    continue
                # fused phase D: u2[w] = dinv * (h1[w] @ W2)
                pt = psum.tile([128, OUT], f32, tag="mm")
                for ch in range(NCH):
                    nc.tensor.matmul(pt[:n, :], h1T[:, ch, wsl],
                                     w2s[:, ch, :], start=(ch == 0),
                                     stop=(ch == NCH - 1))
                nc.scalar.activation(u2res[:n, w, :], pt[:n, :], AF.Copy,
                                     scale=dinvs[:n, w:w + 1])
                if w == SPLITW - 1:
                    store_rng(u2da, u2res, True)
                    nc.gpsimd.collective_compute(
                        "AllGather", mybir.AluOpType.bypass,
                        replica_groups=rg, ins=[u2da[:]], outs=[U2a[:]])
            if MAXP <= 3:
                return nc
            store_rng(u2db, u2res, False)
            nc.gpsimd.collective_compute(
                "AllGather", mybir.AluOpType.bypass, replica_groups=rg,
                ins=[u2db[:]], outs=[U2b[:]])

            # ---- layer 2 aggregation (2 passes) -> out ----
            pass_a(U2a, OUT, u2res)
            for w, n, zt in pass_b(U2b, OUT, b2bc, False, True):
                wsl = slice(w * WS, w * WS + n)
                nc.sync.dma_start(out_p[wsl, :], zt[:n, :])

    return nc


def _assign_gather_queues(nc):
    """Post-schedule queue assignment: queue_num = BIR position % NQ.

    Tile assigns SWDGE DMA-completion semaphore lanes round-robin over the
    *scheduled* order of Pool DMA instructions (lane = pos % 8), ignoring
    queue_num.  Each lane must only ever be incremented by one SWDGE queue
    (decoder shadow-sem rule), so the queue must also be a function of the
    scheduled position: queue = pos % NQ gives queue q the lane set
    {q, q+NQ}.  Build-time round-robin is NOT sufficient because the
    scheduler reorders independent gathers.
    """
    import concourse.mybir as mybir

    pos = 0
    for f in nc.m.functions:
        for bb in f.blocks:
            for inst in bb.instructions:
                if isinstance(inst, mybir.InstDMAGatherAnt):
                    inst.queue_num = pos % NQ
                    pos += 1
                elif (getattr(inst, "engine", None) == mybir.EngineType.Pool
                      and isinstance(inst, (mybir.InstDMACopy,
                                            mybir.InstDMAScatterAddAnt))):
                    raise AssertionError(
                        "unexpected Pool DMA inst would shift SWDGE sem lanes")
    return pos


def run(cfg, inputs, sim=False, trace=False):
    from concourse.bass_utils import run_bass_kernel_spmd

    in_maps, T, b1nz, b2nz = _prepare(
        cfg, inputs["x"], inputs["edge_index"], inputs["W1"], inputs["b1"],
        inputs["W2"], inputs["b2"])
    nc = build_program(cfg, T, b1nz, b2nz)
    nc.finalize()
    _assign_gather_queues(nc)
    core_ids = list(range(cfg.P))
    if sim:
        from concourse import bass_interp
        ms = bass_interp.MultiCoreSim(nc, cfg.P)
        for c in core_ids:
            for k, v in in_maps[c].items():
                ms.cores[c].tensor(k)[:] = v
        ms.simulate()
        outs = [np.array(ms.cores[c].tensor("out")) for c in core_ids]
        return np.concatenate(outs, axis=0), None
    res = run_bass_kernel_spmd(nc, in_maps, core_ids, trace=trace)
    outs = [np.asarray(res.results[c]["out"]) for c in core_ids]
    return np.concatenate(outs, axis=0), res


def kernel(x, edge_index, W1, b1, W2, b2):
    out, _ = run(FULL, dict(x=x, edge_index=edge_index, W1=W1, b1=b1,
                            W2=W2, b2=b2))
    return out
